# revision 37
# baseline (speedup 1.0000x reference)
"""CapsuleLayer dynamic-routing kernel for 8 Trainium2 NeuronCores.

Strategy: data-parallel over batch (64 / 8 cores = 8 batches per core, no
collectives). Inside each core:
  - u_hat[b,i,j,e] built once via TensorE matmuls with a host-prepacked
    block-diagonal u operand; kept SBUF-resident bf16, partition layout
    (b4, i32) per quad, free (quad, blk, j*16+e).
  - s1 (uniform c) folded into the build as tiny 8-col matmuls with the
    W tile as stationary (accumulated in 4 PSUM slices over all blocks).
  - Routing iters 2,3: agreement update via DVE mul (bf16 2x mode) +
    contiguous-halving adds split DVE/Pool; softmax via one ACT exp per
    chunk + DVE reduce/recip; c scattered into the masked-CM operand by
    SBUF->SBUF DMA; weighted sum s via TensorE masked-CM matmuls with
    diag-extraction (mask + selector matmul).
"""

import sys

sys.path.insert(0, "/opt/trn_rl_repo")

import numpy as np
import ml_dtypes

B, NI, DI, NO, DO = 64, 2048, 8, 32, 16
NC_CORES = 8
BL = B // NC_CORES          # 8 batches per core
JE = NO * DO                # 512
NBLK = NI // 32             # 64 blocks of 32 input capsules
NQ = 2                      # 2 quads of 4 batches
CH = 8                      # blocks per bupd chunk
NCHUNK = NBLK // CH         # 8
EPS = 1e-7
BF16 = ml_dtypes.bfloat16

_cache = {}


def _build_program():
    import concourse.bass as bass
    import concourse.bacc as bacc
    import concourse.mybir as mybir
    import concourse.tile as tile

    f32 = mybir.dt.float32
    bf16 = mybir.dt.bfloat16

    nc = bacc.Bacc("TRN2", target_bir_lowering=False, debug=False,
                   num_devices=NC_CORES)

    wu_d = nc.dram_tensor("wu", [NBLK, 128, 2 * JE + NQ * 256 + 16], bf16,
                          kind="ExternalInput")
    dm_d = nc.dram_tensor("diagmask", [128, JE], bf16, kind="ExternalInput")
    id_d = nc.dram_tensor("ident", [128, 128], f32, kind="ExternalInput")
    sq_d = nc.dram_tensor("selq", [128, NQ, BL], bf16, kind="ExternalInput")
    vout_d = nc.dram_tensor("v_out", [BL, JE], f32, kind="ExternalOutput")

    with tile.TileContext(nc) as tc:
        with (
            tc.tile_pool(name="singles", bufs=1) as singles,
            tc.tile_pool(name="wstream", bufs=4) as wpool,
            tc.tile_pool(name="pbuf", bufs=1) as ppool,
            tc.tile_pool(name="t1buf", bufs=2) as t1pool,
            tc.tile_pool(name="tbuf", bufs=1) as tpool,
            tc.tile_pool(name="cdh", bufs=2) as cdpool,
            tc.tile_pool(name="small", bufs=1) as spool,
            tc.tile_pool(name="build_ps", bufs=3, space="PSUM") as build_ps,
            tc.tile_pool(name="junk_ps", bufs=1, space="PSUM") as junk_ps,
            tc.tile_pool(name="s1_ps", bufs=1, space="PSUM") as s1_ps_pool,
            tc.tile_pool(name="spass_ps", bufs=2, space="PSUM") as spass_ps,
            tc.tile_pool(name="s_ps", bufs=1, space="PSUM") as s_ps_pool,
        ):
            # ---- persistent SBUF state ----
            UH = singles.tile([128, NQ, NBLK, JE], bf16)      # 128 KiB/part
            LOG = singles.tile([128, NQ, NBLK, NO], bf16)     # 8 KiB/part
            CM0 = singles.tile([128, NBLK, 128], bf16)        # 16 KiB/part
            CM1 = singles.tile([128, NBLK, 128], bf16)        # 16 KiB/part
            DM = singles.tile([128, JE], bf16)
            IDT = singles.tile([128, 128], f32)
            vb1 = singles.tile([32, 128], bf16)
            SQ = singles.tile([128, NQ, BL], bf16)
            VREP = singles.tile([128, JE], bf16)
            s1T = singles.tile([128, 4, BL], f32)
            s_sb = singles.tile([BL, JE], f32)
            vb_sb = singles.tile([BL, JE], bf16)

            nc.sync.dma_start(out=DM[:, :], in_=dm_d[:, :])
            nc.sync.dma_start(out=IDT[:, :], in_=id_d[:, :])
            nc.sync.dma_start(out=SQ[:, :, :], in_=sq_d[:, :, :])
            nc.gpsimd.memset(LOG[:, :, :, :], 0.0)
            nc.gpsimd.memset(CM0[:, :, :], 0.0)
            nc.gpsimd.memset(CM1[:, :, :], 0.0)

            # 4 persistent PSUM accumulators for s1 (start=False onto zeros)
            s1p = s1_ps_pool.tile([128, 4, BL], f32, tag="s1p")
            nc.vector.memset(s1p[:, :, :], 0.0)

            # ---- phase 1: build u_hat (+ s1 micro-matmuls) ----
            WCOLS = 2 * JE + NQ * 256 + 16
            for blk in range(NBLK):
                    w_t = wpool.tile([128, WCOLS], bf16, tag="w")
                    nc.sync.dma_start(out=w_t[:, :], in_=wu_d[blk, :, :])
                    for q in range(NQ):
                        o = 2 * JE + q * 256
                        ps = build_ps.tile([128, JE], f32, tag="bps")
                        nc.tensor.matmul(ps[:, :], w_t[:, o:o + 128],
                                         w_t[:, 0:JE],
                                         start=True, stop=False)
                        nc.tensor.matmul(ps[:, :], w_t[:, o + 128:o + 256],
                                         w_t[:, JE:2 * JE],
                                         start=False, stop=True)
                        r = (blk * NQ + q) % 5
                        if r < 3:
                            nc.scalar.copy(UH[:, q, blk, :], ps[:, :])
                        else:
                            nc.vector.tensor_copy(UH[:, q, blk, :], ps[:, :])
                    # PE keep-warm: dummy matmul so the p-state ramp never
                    # resets while waiting on the wu stream.
                    junk = junk_ps.tile([128, JE], f32, tag="junk")
                    nc.tensor.matmul(junk[:, :], w_t[:, 0:128],
                                     w_t[:, 0:JE], start=True, stop=True,
                                     skip_group_check=True)
                    # s1 fold: s1T[(j,e)slice o, b] += w_half_slice^T @ u32
                    for h in range(2):
                        for o4 in range(4):
                            nc.tensor.matmul(
                                s1p[:, o4, :],
                                w_t[:, JE * h + 128 * o4:
                                     JE * h + 128 * (o4 + 1)],
                                w_t[:, 2 * JE + NQ * 256 + 8 * h:
                                     2 * JE + NQ * 256 + 8 * (h + 1)],
                                start=False,
                                stop=(blk == NBLK - 1 and h == 1),
                                skip_group_check=True,
                            )

            # s1: PE-transpose [128,(o4,b)=32] -> [32,128] into the junk
            # psum bank, then squash directly in that layout -> vb1.
            nc.vector.tensor_copy(s1T[:, :, :], s1p[:, :, :])
            junk32 = junk_ps.tile([128, JE], f32, tag="junk")
            nc.tensor.matmul(junk32[0:32, 0:128],
                             s1T.rearrange("p a b -> p (a b)"),
                             IDT[:, :], is_transpose=True,
                             start=True, stop=True, skip_group_check=True)
            s1v = junk32[0:32, 0:128]
            SQ32 = spool.tile([32, 128], bf16, tag="SQ32")
            nc.scalar.square(SQ32[:, :], s1v)
            N232 = spool.tile([32, 8], f32, tag="N232")
            nc.vector.tensor_reduce(
                out=N232[:, :],
                in_=SQ32.rearrange("p (j e) -> p j e", e=DO),
                axis=mybir.AxisListType.X, op=mybir.AluOpType.add)
            SR32 = spool.tile([32, 8], f32, tag="SR32")
            nc.vector.tensor_scalar_add(SR32[:, :], N232[:, :], EPS)
            nc.scalar.activation(SR32[:, :], SR32[:, :],
                                 mybir.ActivationFunctionType.Sqrt)
            T132 = spool.tile([32, 8], f32, tag="T132")
            nc.vector.tensor_scalar_add(T132[:, :], N232[:, :], 1.0)
            nc.vector.tensor_mul(T132[:, :], T132[:, :], SR32[:, :])
            nc.vector.reciprocal(T132[:, :], T132[:, :])
            nc.vector.tensor_mul(N232[:, :], N232[:, :], T132[:, :])
            f32b = N232.unsqueeze(2).broadcast_to([32, 8, DO])
            nc.vector.tensor_mul(
                vb1.rearrange("p (j e) -> p j e", e=DO),
                bass.AP(tensor=s1v.tensor, offset=s1v.offset,
                        ap=[list(s1v.ap[0]), [DO, 8], [1, DO]]),
                f32b)
            # unpack vb1[o*8+b, (j8,e)] -> vb_sb[b, (j,e)] (8 row DMAs)
            for b in range(BL):
                vsrc = vb1[b:b + 1, :]
                vdst = vb_sb[b:b + 1, :]
                eng = (nc.sync, nc.scalar)[b % 2]
                eng.dma_start(
                    out=bass.AP(tensor=vdst.tensor, offset=vdst.offset,
                                ap=[list(vdst.ap[0]), [128, 4], [1, 128]]),
                    in_=bass.AP(tensor=vsrc.tensor, offset=vsrc.offset,
                                ap=[[vsrc.ap[0][0] * 8, 4], [1, 128]]),
                )

            # ---- routing iterations ----
            for t in (2, 3):
                if True:
                    # agreement update with v_(t-1) + softmax -> c_t -> CM,
                    # then the s-pass, per quad (PE overlaps next quad's
                    # vector work).
                    s_ps = s_ps_pool.tile([BL, JE], f32, tag="sps")
                    for q in range(NQ):
                        vq = vb_sb[4 * q:4 * q + 4, :]
                        vsrc = bass.AP(
                            tensor=vq.tensor,
                            offset=vq.offset,
                            ap=[list(vq.ap[0]), [0, 32], list(vq.ap[1])],
                        )
                        nc.sync.dma_start(out=VREP[:, :], in_=vsrc)
                        cm_q = CM0 if q == 0 else CM1
                        CDHs = {}

                        def cd_of(ch):
                            pair = ch // 2
                            if pair not in CDHs:
                                CDHs[pair] = cdpool.tile(
                                    [128, 2 * CH, NO], bf16, tag="cdh",
                                    name=f"cdh_{t}_{q}_{pair}")
                            return CDHs[pair][:, (ch % 2) * CH:
                                              (ch % 2 + 1) * CH, :]

                        def pass_b(ch):
                            # softmax normalize: Z on Pool, recip DVE,
                            # scale on Pool
                            cd = cd_of(ch)
                            Z = spool.tile([128, CH], f32, tag="Z")
                            nc.vector.tensor_reduce(
                                out=Z[:, :], in_=cd,
                                axis=mybir.AxisListType.X,
                                op=mybir.AluOpType.add)
                            RZ = spool.tile([128, CH], f32, tag="RZ")
                            nc.vector.reciprocal(RZ[:, :], Z[:, :])
                            rzb = RZ.unsqueeze(2).broadcast_to(
                                [128, CH, NO])
                            nc.gpsimd.tensor_mul(cd, cd, rzb)

                        def scatter(pair):
                            hb = slice(pair * 2 * CH, (pair + 1) * 2 * CH)
                            for b4 in range(4):
                                rows = slice(32 * b4, 32 * b4 + 32)
                                nc.sync.dma_start(
                                    out=cm_q[rows, hb,
                                             32 * b4:32 * b4 + 32],
                                    in_=CDHs[pair][rows, :, :],
                                )

                        sp = spass_ps.tile([128, JE], f32, tag="spq")

                        def spass_pair(pair):
                            for blk in range(pair * 2 * CH,
                                             (pair + 1) * 2 * CH):
                                nc.tensor.matmul(
                                    sp[:, :],
                                    cm_q[:, blk, :],
                                    UH[:, q, blk, :],
                                    start=(blk == 0),
                                    stop=(blk == NBLK - 1),
                                )

                        for ch in range(NCHUNK):
                            blks = slice(ch * CH, (ch + 1) * CH)
                            P = ppool.tile([128, CH, JE], bf16, tag="P")
                            vrb = VREP[:, :].unsqueeze(1).broadcast_to(
                                [128, CH, JE])
                            nc.vector.tensor_mul(
                                P[:, :, :], UH[:, q, blks, :], vrb)
                            Pv = P.rearrange("p c (j e) -> p c j e", e=DO)
                            T1 = t1pool.tile([128, CH, NO, 8], bf16,
                                             tag="T1")
                            nc.vector.tensor_add(
                                T1[:, :, :, :], Pv[:, :, :, 0:8],
                                Pv[:, :, :, 8:16])
                            T2 = tpool.tile([128, CH, NO, 4], bf16, tag="T2")
                            nc.vector.tensor_add(
                                T2[:, :, :, :], T1[:, :, :, 0:4],
                                T1[:, :, :, 4:8])
                            for k3 in range(4):
                                nc.gpsimd.tensor_add(
                                    LOG[:, q, blks, :], LOG[:, q, blks, :],
                                    T2[:, :, :, k3])
                            nc.scalar.activation(
                                cd_of(ch), LOG[:, q, blks, :],
                                mybir.ActivationFunctionType.Exp)
                            if ch >= 2:
                                pass_b(ch - 2)
                                if ch % 2 == 1:
                                    scatter(ch // 2 - 1)
                                    if ch == 7:
                                        spass_pair(0)
                                        spass_pair(1)
                        pass_b(6)
                        pass_b(7)
                        scatter(3)
                        spass_pair(2)
                        spass_pair(3)

                        ME = spool.tile([128, JE], bf16, tag="ME")
                        nc.vector.tensor_mul(ME[:, :], sp[:, :], DM[:, :])
                        nc.tensor.matmul(
                            s_ps[:, :], SQ[:, q, :], ME[:, :],
                            start=(q == 0), stop=(q == NQ - 1),
                        )
                    nc.vector.tensor_copy(s_sb[:, :], s_ps[:, :])

                # squash: f = n2 / ((1+n2) * sqrt(n2+eps)); v = s * f
                SQT_t = spool.tile([128, JE], bf16, tag="ME")
                SQT = SQT_t[0:BL, :]
                nc.vector.tensor_mul(SQT[:, :], s_sb[:, :], s_sb[:, :])
                N2 = spool.tile([BL, NO], f32, tag="N2")
                nc.vector.tensor_reduce(
                    out=N2[:, :],
                    in_=SQT.rearrange("p (j e) -> p j e", e=DO),
                    axis=mybir.AxisListType.X,
                    op=mybir.AluOpType.add,
                )
                SRT = spool.tile([BL, NO], f32, tag="SRT")
                nc.vector.tensor_scalar_add(SRT[:, :], N2[:, :], EPS)
                nc.scalar.activation(SRT[:, :], SRT[:, :],
                                     mybir.ActivationFunctionType.Sqrt)
                T1s = spool.tile([BL, NO], f32, tag="T1s")
                nc.vector.tensor_scalar_add(T1s[:, :], N2[:, :], 1.0)
                nc.vector.tensor_mul(T1s[:, :], T1s[:, :], SRT[:, :])
                nc.vector.reciprocal(T1s[:, :], T1s[:, :])
                nc.vector.tensor_mul(N2[:, :], N2[:, :], T1s[:, :])
                fb = N2.unsqueeze(2).broadcast_to([BL, NO, DO])
                if t < 3:
                    nc.vector.tensor_mul(
                        vb_sb.rearrange("p (j e) -> p j e", e=DO),
                        s_sb.rearrange("p (j e) -> p j e", e=DO), fb)
                else:
                    nc.vector.tensor_mul(
                        s_sb.rearrange("p (j e) -> p j e", e=DO),
                        s_sb.rearrange("p (j e) -> p j e", e=DO), fb)
                    nc.sync.dma_start(out=vout_d[:, :], in_=s_sb[:, :])

    nc.compile()
    return nc


def _host_prep(u, W):
    """Prepack operands. Returns per-core input maps."""
    # W: [NI, NO, DI, DO] -> w32[blk, h, i*4+dl, j*16+e] = W[blk*32+i, j, 4h+dl, e]
    w32 = (
        W.reshape(NBLK, 32, NO, 2, 4, DO)       # blk, i, j, h, dl, e
        .transpose(0, 1, 4, 3, 2, 5)            # blk, i, dl, h, j, e
        .reshape(NBLK, 128, 2 * JE)
        .astype(BF16)
    )
    # u block-diagonal: ubd[core][blk, h, q, i*4+dl, b*32+i] = u[core*8+4q+b, blk*32+i, 4h+dl]
    ur = u.reshape(NC_CORES, NQ, 4, NBLK, 32, 2, 4)  # c, q, b, blk, i, h, dl
    ubd = np.zeros((NC_CORES, NBLK, NQ, 128, 2, 128), dtype=BF16)
    for i in range(32):
        blkslice = ur[:, :, :, :, i, :, :].transpose(0, 3, 1, 5, 4, 2)
        ubd[:, :, :, 4 * i:4 * i + 4, :, i::32] = blkslice.astype(BF16)
    ubd = ubd.reshape(NC_CORES, NBLK, NQ, 128, 256)
    ubd = ubd.transpose(0, 1, 3, 2, 4).reshape(NC_CORES, NBLK, 128, NQ * 256)
    # u32 dense (scaled 1/NO), appended per-block to wu:
    # wu[c][blk, i*4+dl, 1536 + 8h + b] = u[core*8+b, blk*32+i, 4h+dl]/32
    u32 = (
        u.reshape(NC_CORES, BL, NBLK, 32, 2, 4)  # c, b, blk, i, h, dl
        .transpose(0, 2, 3, 5, 4, 1)             # c, blk, i, dl, h, b
        .reshape(NC_CORES, NBLK, 128, 2 * BL)
        / NO
    ).astype(BF16)
    wu = np.concatenate(
        [np.broadcast_to(w32[None], (NC_CORES,) + w32.shape), ubd, u32],
        axis=3)
    # diag mask: dm[p, j*16+e] = (j == p % 32)
    pj = np.arange(128) % 32
    dm = (np.arange(NO)[None, :] == pj[:, None]).astype(np.float32)
    dm = np.repeat(dm, DO, axis=1)
    dm = dm.reshape(128, NO, DO).reshape(128, JE).astype(BF16)
    ident = np.eye(128, dtype=np.float32)
    # selector: sq[p, q, b'] = (b' == 4q + p//32)
    sq = np.zeros((128, NQ, BL), dtype=np.float32)  # cast to bf16 below
    for q in range(NQ):
        for p in range(128):
            sq[p, q, 4 * q + p // 32] = 1.0
    return wu, dm, sq.astype(BF16), ident


def kernel(u, W):
    from concourse.bass_utils import run_bass_kernel_spmd

    key = "prog"
    if key not in _cache:
        _cache[key] = _build_program()
    nc = _cache[key]

    wu, dm, sq, ident = _host_prep(np.asarray(u, np.float32),
                                   np.asarray(W, np.float32))
    in_maps = [
        {"wu": wu[c], "diagmask": dm, "selq": sq, "ident": ident}
        for c in range(NC_CORES)
    ]
    res = run_bass_kernel_spmd(nc, in_maps, list(range(NC_CORES)))
    out = np.concatenate([res.results[c]["v_out"] for c in range(NC_CORES)],
                         axis=0)
    return out.reshape(B, NO, DO).astype(np.float32)


# revision 38
# speedup vs baseline: 1.0242x; 1.0242x over previous
"""CapsuleLayer dynamic-routing kernel for 8 Trainium2 NeuronCores.

Strategy: data-parallel over batch (64 / 8 cores = 8 batches per core, no
collectives). Inside each core:
  - u_hat[b,i,j,e] built once via TensorE matmuls with a host-prepacked
    block-diagonal u operand; kept SBUF-resident bf16, partition layout
    (b4, i32) per quad, free (quad, blk, j*16+e).
  - s1 (uniform c) folded into the build as tiny 8-col matmuls with the
    W tile as stationary (accumulated in 4 PSUM slices over all blocks).
  - Routing iters 2,3: agreement update via DVE mul (bf16 2x mode) +
    contiguous-halving adds split DVE/Pool; softmax via one ACT exp per
    chunk + DVE reduce/recip; c scattered into the masked-CM operand by
    SBUF->SBUF DMA; weighted sum s via TensorE masked-CM matmuls with
    diag-extraction (mask + selector matmul).
"""

import sys

sys.path.insert(0, "/opt/trn_rl_repo")

import numpy as np
import ml_dtypes

B, NI, DI, NO, DO = 64, 2048, 8, 32, 16
NC_CORES = 8
BL = B // NC_CORES          # 8 batches per core
JE = NO * DO                # 512
NBLK = NI // 32             # 64 blocks of 32 input capsules
NQ = 2                      # 2 quads of 4 batches
CH = 8                      # blocks per bupd chunk
NCHUNK = NBLK // CH         # 8
EPS = 1e-7
BF16 = ml_dtypes.bfloat16

_cache = {}


def _build_program():
    import concourse.bass as bass
    import concourse.bacc as bacc
    import concourse.mybir as mybir
    import concourse.tile as tile

    f32 = mybir.dt.float32
    bf16 = mybir.dt.bfloat16

    nc = bacc.Bacc("TRN2", target_bir_lowering=False, debug=False,
                   num_devices=NC_CORES)

    wu_d = nc.dram_tensor("wu", [NBLK, 128, 2 * JE + NQ * 256 + 16], bf16,
                          kind="ExternalInput")
    dm_d = nc.dram_tensor("diagmask", [128, JE], bf16, kind="ExternalInput")
    id_d = nc.dram_tensor("ident", [128, 128], f32, kind="ExternalInput")
    sq_d = nc.dram_tensor("selq", [128, NQ, BL], bf16, kind="ExternalInput")
    vout_d = nc.dram_tensor("v_out", [BL, JE], f32, kind="ExternalOutput")

    with tile.TileContext(nc) as tc:
        with (
            tc.tile_pool(name="singles", bufs=1) as singles,
            tc.tile_pool(name="wstream", bufs=4) as wpool,
            tc.tile_pool(name="pbuf", bufs=1) as ppool,
            tc.tile_pool(name="t1buf", bufs=2) as t1pool,
            tc.tile_pool(name="tbuf", bufs=1) as tpool,
            tc.tile_pool(name="cdh", bufs=2) as cdpool,
            tc.tile_pool(name="small", bufs=1) as spool,
            tc.tile_pool(name="build_ps", bufs=3, space="PSUM") as build_ps,
            tc.tile_pool(name="junk_ps", bufs=1, space="PSUM") as junk_ps,
            tc.tile_pool(name="s1_ps", bufs=1, space="PSUM") as s1_ps_pool,
            tc.tile_pool(name="spass_ps", bufs=2, space="PSUM") as spass_ps,
            tc.tile_pool(name="s_ps", bufs=1, space="PSUM") as s_ps_pool,
        ):
            # ---- persistent SBUF state ----
            UH = singles.tile([128, NQ, NBLK, JE], bf16)      # 128 KiB/part
            LOG = singles.tile([128, NQ, NBLK, NO], bf16)     # 8 KiB/part
            CM0 = singles.tile([128, NBLK, 128], bf16)        # 16 KiB/part
            CM1 = singles.tile([128, NBLK, 128], bf16)        # 16 KiB/part
            DM = singles.tile([128, JE], bf16)
            IDT = singles.tile([128, 128], f32)
            vb1 = singles.tile([32, 128], bf16)
            SQ = singles.tile([128, NQ, BL], bf16)
            VREP = singles.tile([128, JE], bf16)
            s1T = singles.tile([128, 4, BL], f32)
            s_sb = singles.tile([BL, JE], f32)
            vb_sb = singles.tile([BL, JE], bf16)

            nc.sync.dma_start(out=DM[:, :], in_=dm_d[:, :])
            nc.sync.dma_start(out=IDT[:, :], in_=id_d[:, :])
            nc.sync.dma_start(out=SQ[:, :, :], in_=sq_d[:, :, :])
            nc.gpsimd.memset(LOG[:, :, :, :], 0.0)
            nc.gpsimd.memset(CM0[:, :, :], 0.0)
            nc.gpsimd.memset(CM1[:, :, :], 0.0)

            # 4 persistent PSUM accumulators for s1 (start=False onto zeros)
            s1p = s1_ps_pool.tile([128, 4, BL], f32, tag="s1p")
            nc.vector.memset(s1p[:, :, :], 0.0)

            # ---- phase 1: build u_hat (+ s1 micro-matmuls) ----
            WCOLS = 2 * JE + NQ * 256 + 16
            for blk in range(NBLK):
                    w_t = wpool.tile([128, WCOLS], bf16, tag="w")
                    nc.sync.dma_start(out=w_t[:, :], in_=wu_d[blk, :, :])
                    for q in range(NQ):
                        o = 2 * JE + q * 256
                        ps = build_ps.tile([128, JE], f32, tag="bps")
                        nc.tensor.matmul(ps[:, :], w_t[:, o:o + 128],
                                         w_t[:, 0:JE],
                                         start=True, stop=False)
                        nc.tensor.matmul(ps[:, :], w_t[:, o + 128:o + 256],
                                         w_t[:, JE:2 * JE],
                                         start=False, stop=True)
                        r = (blk * NQ + q) % 5
                        if r < 3:
                            nc.scalar.copy(UH[:, q, blk, :], ps[:, :])
                        else:
                            nc.vector.tensor_copy(UH[:, q, blk, :], ps[:, :])
                    # PE keep-warm: dummy matmul so the p-state ramp never
                    # resets while waiting on the wu stream.
                    junk = junk_ps.tile([128, JE], f32, tag="junk")
                    nc.tensor.matmul(junk[:, :], w_t[:, 0:128],
                                     w_t[:, 0:JE], start=True, stop=True,
                                     skip_group_check=True)
                    # s1 fold: s1T[(j,e)slice o, b] += w_half_slice^T @ u32
                    for h in range(2):
                        for o4 in range(4):
                            nc.tensor.matmul(
                                s1p[:, o4, :],
                                w_t[:, JE * h + 128 * o4:
                                     JE * h + 128 * (o4 + 1)],
                                w_t[:, 2 * JE + NQ * 256 + 8 * h:
                                     2 * JE + NQ * 256 + 8 * (h + 1)],
                                start=False,
                                stop=(blk == NBLK - 1 and h == 1),
                                skip_group_check=True,
                            )

            # s1: PE-transpose [128,(o4,b)=32] -> [32,128] into the junk
            # psum bank, then squash directly in that layout -> vb1.
            nc.vector.tensor_copy(s1T[:, :, :], s1p[:, :, :])
            junk32 = junk_ps.tile([128, JE], f32, tag="junk")
            nc.tensor.matmul(junk32[0:32, 0:128],
                             s1T.rearrange("p a b -> p (a b)"),
                             IDT[:, :], is_transpose=True,
                             start=True, stop=True, skip_group_check=True)
            s1v = junk32[0:32, 0:128]
            SQ32 = spool.tile([32, 128], bf16, tag="SQ32")
            nc.scalar.square(SQ32[:, :], s1v)
            N232 = spool.tile([32, 8], f32, tag="N232")
            nc.vector.tensor_reduce(
                out=N232[:, :],
                in_=SQ32.rearrange("p (j e) -> p j e", e=DO),
                axis=mybir.AxisListType.X, op=mybir.AluOpType.add)
            SR32 = spool.tile([32, 8], f32, tag="SR32")
            nc.vector.tensor_scalar_add(SR32[:, :], N232[:, :], EPS)
            nc.scalar.activation(SR32[:, :], SR32[:, :],
                                 mybir.ActivationFunctionType.Sqrt)
            T132 = spool.tile([32, 8], f32, tag="T132")
            nc.vector.tensor_scalar_add(T132[:, :], N232[:, :], 1.0)
            nc.vector.tensor_mul(T132[:, :], T132[:, :], SR32[:, :])
            nc.vector.reciprocal(T132[:, :], T132[:, :])
            nc.vector.tensor_mul(N232[:, :], N232[:, :], T132[:, :])
            f32b = N232.unsqueeze(2).broadcast_to([32, 8, DO])
            nc.vector.tensor_mul(
                vb1.rearrange("p (j e) -> p j e", e=DO),
                bass.AP(tensor=s1v.tensor, offset=s1v.offset,
                        ap=[list(s1v.ap[0]), [DO, 8], [1, DO]]),
                f32b)
            # unpack vb1[o*8+b, (j8,e)] -> vb_sb[b, (j,e)] (8 row DMAs)
            for b in range(BL):
                vsrc = vb1[b:b + 1, :]
                vdst = vb_sb[b:b + 1, :]
                eng = (nc.sync, nc.scalar)[b % 2]
                eng.dma_start(
                    out=bass.AP(tensor=vdst.tensor, offset=vdst.offset,
                                ap=[list(vdst.ap[0]), [128, 4], [1, 128]]),
                    in_=bass.AP(tensor=vsrc.tensor, offset=vsrc.offset,
                                ap=[[vsrc.ap[0][0] * 8, 4], [1, 128]]),
                )

            # ---- routing iterations ----
            for t in (2, 3):
                if True:
                    # agreement update with v_(t-1) + softmax -> c_t -> CM,
                    # then the s-pass, per quad (PE overlaps next quad's
                    # vector work).
                    s_ps = s_ps_pool.tile([BL, JE], f32, tag="sps")
                    for q in range(NQ):
                        vq = vb_sb[4 * q:4 * q + 4, :]
                        vsrc = bass.AP(
                            tensor=vq.tensor,
                            offset=vq.offset,
                            ap=[list(vq.ap[0]), [0, 32], list(vq.ap[1])],
                        )
                        nc.sync.dma_start(out=VREP[:, :], in_=vsrc)
                        cm_q = CM0 if q == 0 else CM1
                        CDHs = {}

                        def cd_of(ch):
                            pair = ch // 2
                            if pair not in CDHs:
                                CDHs[pair] = cdpool.tile(
                                    [128, 2 * CH, NO], bf16, tag="cdh",
                                    name=f"cdh_{t}_{q}_{pair}")
                            return CDHs[pair][:, (ch % 2) * CH:
                                              (ch % 2 + 1) * CH, :]

                        def pass_b(ch):
                            # softmax normalize: Z on Pool, recip DVE,
                            # scale on Pool
                            cd = cd_of(ch)
                            Z = spool.tile([128, CH], f32, tag="Z")
                            nc.vector.tensor_reduce(
                                out=Z[:, :], in_=cd,
                                axis=mybir.AxisListType.X,
                                op=mybir.AluOpType.add)
                            RZ = spool.tile([128, CH], f32, tag="RZ")
                            nc.vector.reciprocal(RZ[:, :], Z[:, :])
                            for kb in range(CH):
                                nc.scalar.mul(cd[:, kb, :], cd[:, kb, :],
                                              RZ[:, kb:kb + 1])

                        def scatter(pair):
                            hb = slice(pair * 2 * CH, (pair + 1) * 2 * CH)
                            for b4 in range(4):
                                rows = slice(32 * b4, 32 * b4 + 32)
                                nc.sync.dma_start(
                                    out=cm_q[rows, hb,
                                             32 * b4:32 * b4 + 32],
                                    in_=CDHs[pair][rows, :, :],
                                )

                        sp = spass_ps.tile([128, JE], f32, tag="spq")

                        def spass_pair(pair):
                            for blk in range(pair * 2 * CH,
                                             (pair + 1) * 2 * CH):
                                nc.tensor.matmul(
                                    sp[:, :],
                                    cm_q[:, blk, :],
                                    UH[:, q, blk, :],
                                    start=(blk == 0),
                                    stop=(blk == NBLK - 1),
                                )

                        for ch in range(NCHUNK):
                            blks = slice(ch * CH, (ch + 1) * CH)
                            P = ppool.tile([128, CH, JE], bf16, tag="P")
                            vrb = VREP[:, :].unsqueeze(1).broadcast_to(
                                [128, CH, JE])
                            nc.vector.tensor_mul(
                                P[:, :, :], UH[:, q, blks, :], vrb)
                            Pv = P.rearrange("p c (j e) -> p c j e", e=DO)
                            T1 = t1pool.tile([128, CH, NO, 8], bf16,
                                             tag="T1")
                            nc.vector.tensor_add(
                                T1[:, :, :, :], Pv[:, :, :, 0:8],
                                Pv[:, :, :, 8:16])
                            T2 = tpool.tile([128, CH, NO, 4], bf16, tag="T2")
                            t2eng = nc.vector if ch % 2 == 0 else nc.gpsimd
                            t2eng.tensor_add(
                                T2[:, :, :, :], T1[:, :, :, 0:4],
                                T1[:, :, :, 4:8])
                            for k3 in range(4):
                                nc.gpsimd.tensor_add(
                                    LOG[:, q, blks, :], LOG[:, q, blks, :],
                                    T2[:, :, :, k3])
                            nc.scalar.activation(
                                cd_of(ch), LOG[:, q, blks, :],
                                mybir.ActivationFunctionType.Exp)
                            if ch >= 2:
                                pass_b(ch - 2)
                                if ch % 2 == 1:
                                    scatter(ch // 2 - 1)
                                    if ch == 7:
                                        spass_pair(0)
                                        spass_pair(1)
                        pass_b(6)
                        pass_b(7)
                        scatter(3)
                        spass_pair(2)
                        spass_pair(3)

                        ME = spool.tile([128, JE], bf16, tag="ME")
                        nc.vector.tensor_mul(ME[:, :], sp[:, :], DM[:, :])
                        nc.tensor.matmul(
                            s_ps[:, :], SQ[:, q, :], ME[:, :],
                            start=(q == 0), stop=(q == NQ - 1),
                        )
                    nc.vector.tensor_copy(s_sb[:, :], s_ps[:, :])

                # squash: f = n2 / ((1+n2) * sqrt(n2+eps)); v = s * f
                SQT_t = spool.tile([128, JE], bf16, tag="ME")
                SQT = SQT_t[0:BL, :]
                nc.vector.tensor_mul(SQT[:, :], s_sb[:, :], s_sb[:, :])
                N2 = spool.tile([BL, NO], f32, tag="N2")
                nc.vector.tensor_reduce(
                    out=N2[:, :],
                    in_=SQT.rearrange("p (j e) -> p j e", e=DO),
                    axis=mybir.AxisListType.X,
                    op=mybir.AluOpType.add,
                )
                SRT = spool.tile([BL, NO], f32, tag="SRT")
                nc.vector.tensor_scalar_add(SRT[:, :], N2[:, :], EPS)
                nc.scalar.activation(SRT[:, :], SRT[:, :],
                                     mybir.ActivationFunctionType.Sqrt)
                T1s = spool.tile([BL, NO], f32, tag="T1s")
                nc.vector.tensor_scalar_add(T1s[:, :], N2[:, :], 1.0)
                nc.vector.tensor_mul(T1s[:, :], T1s[:, :], SRT[:, :])
                nc.vector.reciprocal(T1s[:, :], T1s[:, :])
                nc.vector.tensor_mul(N2[:, :], N2[:, :], T1s[:, :])
                fb = N2.unsqueeze(2).broadcast_to([BL, NO, DO])
                if t < 3:
                    nc.vector.tensor_mul(
                        vb_sb.rearrange("p (j e) -> p j e", e=DO),
                        s_sb.rearrange("p (j e) -> p j e", e=DO), fb)
                else:
                    nc.vector.tensor_mul(
                        s_sb.rearrange("p (j e) -> p j e", e=DO),
                        s_sb.rearrange("p (j e) -> p j e", e=DO), fb)
                    nc.sync.dma_start(out=vout_d[:, :], in_=s_sb[:, :])

    nc.compile()
    return nc


def _host_prep(u, W):
    """Prepack operands. Returns per-core input maps."""
    # W: [NI, NO, DI, DO] -> w32[blk, h, i*4+dl, j*16+e] = W[blk*32+i, j, 4h+dl, e]
    w32 = (
        W.reshape(NBLK, 32, NO, 2, 4, DO)       # blk, i, j, h, dl, e
        .transpose(0, 1, 4, 3, 2, 5)            # blk, i, dl, h, j, e
        .reshape(NBLK, 128, 2 * JE)
        .astype(BF16)
    )
    # u block-diagonal: ubd[core][blk, h, q, i*4+dl, b*32+i] = u[core*8+4q+b, blk*32+i, 4h+dl]
    ur = u.reshape(NC_CORES, NQ, 4, NBLK, 32, 2, 4)  # c, q, b, blk, i, h, dl
    ubd = np.zeros((NC_CORES, NBLK, NQ, 128, 2, 128), dtype=BF16)
    for i in range(32):
        blkslice = ur[:, :, :, :, i, :, :].transpose(0, 3, 1, 5, 4, 2)
        ubd[:, :, :, 4 * i:4 * i + 4, :, i::32] = blkslice.astype(BF16)
    ubd = ubd.reshape(NC_CORES, NBLK, NQ, 128, 256)
    ubd = ubd.transpose(0, 1, 3, 2, 4).reshape(NC_CORES, NBLK, 128, NQ * 256)
    # u32 dense (scaled 1/NO), appended per-block to wu:
    # wu[c][blk, i*4+dl, 1536 + 8h + b] = u[core*8+b, blk*32+i, 4h+dl]/32
    u32 = (
        u.reshape(NC_CORES, BL, NBLK, 32, 2, 4)  # c, b, blk, i, h, dl
        .transpose(0, 2, 3, 5, 4, 1)             # c, blk, i, dl, h, b
        .reshape(NC_CORES, NBLK, 128, 2 * BL)
        / NO
    ).astype(BF16)
    wu = np.concatenate(
        [np.broadcast_to(w32[None], (NC_CORES,) + w32.shape), ubd, u32],
        axis=3)
    # diag mask: dm[p, j*16+e] = (j == p % 32)
    pj = np.arange(128) % 32
    dm = (np.arange(NO)[None, :] == pj[:, None]).astype(np.float32)
    dm = np.repeat(dm, DO, axis=1)
    dm = dm.reshape(128, NO, DO).reshape(128, JE).astype(BF16)
    ident = np.eye(128, dtype=np.float32)
    # selector: sq[p, q, b'] = (b' == 4q + p//32)
    sq = np.zeros((128, NQ, BL), dtype=np.float32)  # cast to bf16 below
    for q in range(NQ):
        for p in range(128):
            sq[p, q, 4 * q + p // 32] = 1.0
    return wu, dm, sq.astype(BF16), ident


def kernel(u, W):
    from concourse.bass_utils import run_bass_kernel_spmd

    key = "prog"
    if key not in _cache:
        _cache[key] = _build_program()
    nc = _cache[key]

    wu, dm, sq, ident = _host_prep(np.asarray(u, np.float32),
                                   np.asarray(W, np.float32))
    in_maps = [
        {"wu": wu[c], "diagmask": dm, "selq": sq, "ident": ident}
        for c in range(NC_CORES)
    ]
    res = run_bass_kernel_spmd(nc, in_maps, list(range(NC_CORES)))
    out = np.concatenate([res.results[c]["v_out"] for c in range(NC_CORES)],
                         axis=0)
    return out.reshape(B, NO, DO).astype(np.float32)
